# revision 29
# baseline (speedup 1.0000x reference)
"""2-layer GCN (GCNConv -> ReLU -> GCNConv -> ReLU) on 8 Trainium2 NeuronCores.

Math (per layer, following PyG GCNConv):
    out = D^-1/2 (A + I) D^-1/2 (x @ W) + b
We exploit associativity so the sparse aggregation always runs on 128 features:
    layer1: h1 = relu( (A_norm @ x) @ W1 + b1 )          (aggregate first)
    layer2: out = relu( A_norm @ (h1 @ W2) + b2 )        (transform first)
Per-edge weight norm_e = dinv[src]*dinv[dst] makes the weighted segment-sum
exactly the off-diagonal part of A_norm; the self-loop (diagonal dinv^2) term
is applied densely (layer1: identity matmul of host-precomputed (dinv^2 x)^T;
layer2: matmul of the SBUF-kept row-major t2 against a block-diagonal dinv^2
table), so no self-loop edges enter the gather stream.

Layer-1 gathers are PAIRED: a greedy matching (host-side, per core) pairs up
edges whose sources co-occur in the same 128-dst block, and the per-core x
replica is laid out as a paired table [50176, 256] so one 512-byte gather
descriptor fetches two bf16 source rows that serve two edges.  512B
descriptors also dodge the <512B small-descriptor penalty.  Unpaired edges
use one half of their entry's fetch.  Layer 2 gathers the AllGather'd
t2 = h1@W2 with plain 256-byte descriptors (pairing t2 would require
permuting the shard write order for ~no net gain).

Device algorithm per 128-node destination block:
    - dma_gather fetches pair-entries  -> M [128slot, C, 256f]
    - DVE builds S_h[slot, i] = (iota[i] == dst_rel_h[slot]) * norm_h[slot]
      per (chunk, half) that has edges
    - PE accumulates  psum[f, i] += M_half^T @ S_h  over the block's chunks
giving the aggregated block transposed ([feat, dst]), which feeds the dense
transforms without any transpose; PE transpose mode emits row-major tiles.
"""

import hashlib
import math

import ml_dtypes
import numpy as np

P = 128
NCORES = 8
D_IN, D_HID, D_OUT = 128, 256, 128
GB = 4  # dst blocks per dense group (psum free dim = GB*128 <= 512)
BANK = 25088  # int16-addressable rows per gather-table view (layer 2)
NBANK = 4
NPAIR_BANK = 25088  # pair-entries per bank view (layer 1)
NBANKP = 2
MAXC = 8  # chunks per dma_gather call (1024 descs = SWDGE ring capacity)

_CACHE: dict = {}


def _build(meta, timing_variant=False):
    import concourse.bacc as bacc
    import concourse.mybir as mybir
    import concourse.tile as tile
    from concourse.masks import make_identity

    FP = mybir.dt.float32
    BF = mybir.dt.bfloat16
    R32 = mybir.dt.float32r
    I16 = mybir.dt.int16
    AF = mybir.ActivationFunctionType
    ALU = mybir.AluOpType

    OWN = meta["OWN"]
    n_blocks = meta["n_blocks"]
    n_rows = OWN * NCORES
    n_pairs = n_rows // 2
    # ---- layer 1 (paired) bookkeeping ----
    Ku = meta["Ku"]            # [n_blocks, 2] chunks per cell
    ucol = meta["ucol"]        # [n_blocks, 2, maxK] S-col or -1
    vcol = meta["vcol"]
    cumKu = np.zeros((NBANKP, n_blocks + 1), np.int64)
    for j in range(NBANKP):
        for b in range(n_blocks):
            cumKu[j, b + 1] = cumKu[j, b] + Ku[b][j]
    Cju = [int(cumKu[j, n_blocks]) for j in range(NBANKP)]
    CHS1 = meta["CHS1"]        # total layer-1 S columns
    # ---- layer 2 bookkeeping ----
    Kbj = meta["Kbj"]          # [n_blocks, NBANK]
    chunk_base = np.zeros((n_blocks, NBANK), np.int64)
    acc = 0
    for b in range(n_blocks):
        for j in range(NBANK):
            chunk_base[b, j] = acc
            acc += Kbj[b][j]
    CH2 = acc
    cumK = np.zeros((NBANK, n_blocks + 1), np.int64)
    for j in range(NBANK):
        for b in range(n_blocks):
            cumK[j, b + 1] = cumK[j, b] + Kbj[b][j]
    Cj = [int(cumK[j, n_blocks]) for j in range(NBANK)]
    CHS = CHS1 + CH2

    NSWQ = 4
    nc = bacc.Bacc("TRN2", debug=False, num_devices=NCORES, num_swdge_queues=NSWQ)

    xp_d = nc.dram_tensor("xpair", [n_pairs, 2 * D_IN], BF, kind="ExternalInput")
    xots_d = nc.dram_tensor("xots", [P, OWN], BF, kind="ExternalInput")
    ddcol_d = nc.dram_tensor("ddcol", [P, OWN // P], FP, kind="ExternalInput")
    w1_d = nc.dram_tensor("w1", [D_IN, D_HID], FP, kind="ExternalInput")
    w2_d = nc.dram_tensor("w2", [D_HID, D_OUT], FP, kind="ExternalInput")
    b1_d = nc.dram_tensor("b1h", [P, 2], FP, kind="ExternalInput")
    b2_d = nc.dram_tensor("b2c", [P, 1], FP, kind="ExternalInput")
    iota_d = nc.dram_tensor("iota", [P, P], BF, kind="ExternalInput")
    identb_d = nc.dram_tensor("identb", [P, P], BF, kind="ExternalInput")
    dr_d = nc.dram_tensor("dst_rel", [P, CHS], FP, kind="ExternalInput")
    nm_d = nc.dram_tensor("norm", [P, CHS], FP, kind="ExternalInput")
    idx1_ds = [
        nc.dram_tensor(f"idx1_{j}", [P, max(Cju[j], 1) * 8], I16, kind="ExternalInput")
        for j in range(NBANKP)
    ]
    idx2_ds = [
        nc.dram_tensor(f"idx2_{j}", [P, max(Cj[j], 1) * 8], I16, kind="ExternalInput")
        for j in range(NBANK)
    ]
    out_d = nc.dram_tensor("out", [OWN, D_OUT], FP, kind="ExternalOutput")
    t2_own = nc.dram_tensor("t2_own", [OWN, D_OUT], BF)
    t2_cat = nc.dram_tensor("t2_cat", [n_rows, D_OUT], BF, addr_space="Shared")

    ngroups = (n_blocks + GB - 1) // GB

    with tile.TileContext(nc) as tc:
        with (
            tc.tile_pool(name="const", bufs=1) as constp,
            tc.tile_pool(name="mp1", bufs=6) as mp1,
            tc.tile_pool(name="mp2", bufs=6) as mp2,
            tc.tile_pool(name="sp", bufs=10) as sp,
            tc.tile_pool(name="xs", bufs=2) as xs,
            tc.tile_pool(name="dd", bufs=2) as ddp,
            tc.tile_pool(name="aggs", bufs=3) as aggs,
            tc.tile_pool(name="hs", bufs=2) as hs,
            tc.tile_pool(name="t2s", bufs=3) as t2s,
            tc.tile_pool(name="zs", bufs=3) as zs,
            tc.tile_pool(name="outs", bufs=6) as outs,
            tc.tile_pool(name="aggp", bufs=2, space="PSUM") as aggp,
            tc.tile_pool(name="dps", bufs=2, space="PSUM") as dps,
            tc.tile_pool(name="tpp", bufs=2, space="PSUM") as tpp,
        ):
            iota_t = constp.tile([P, P], BF, tag="iota")
            nc.sync.dma_start(iota_t[:, :], iota_d[:, :])
            ident = constp.tile([P, P], FP, tag="ident")
            make_identity(nc, ident[:, :])
            identb = constp.tile([P, P], BF, tag="identb")
            nc.sync.dma_start(identb[:, :], identb_d[:, :])
            w1a_f = constp.tile([P, P], FP, tag="w1a_f")
            nc.sync.dma_start(w1a_f[:, :], w1_d[:, 0:P])
            w1b_f = constp.tile([P, P], FP, tag="w1b_f")
            nc.sync.dma_start(w1b_f[:, :], w1_d[:, P : 2 * P])
            w2a_f = constp.tile([P, P], FP, tag="w2a_f")
            nc.sync.dma_start(w2a_f[:, :], w2_d[0:P, :])
            w2b_f = constp.tile([P, P], FP, tag="w2b_f")
            nc.sync.dma_start(w2b_f[:, :], w2_d[P : 2 * P, :])
            w1a = constp.tile([P, P], R32, tag="w1a")
            nc.vector.tensor_copy(w1a[:, :], w1a_f[:, :])
            w1b = constp.tile([P, P], R32, tag="w1b")
            nc.vector.tensor_copy(w1b[:, :], w1b_f[:, :])
            w2a = constp.tile([P, P], R32, tag="w2a")
            nc.vector.tensor_copy(w2a[:, :], w2a_f[:, :])
            w2b = constp.tile([P, P], R32, tag="w2b")
            nc.vector.tensor_copy(w2b[:, :], w2b_f[:, :])
            b1t = constp.tile([P, 2], FP, tag="b1")
            nc.sync.dma_start(b1t[:, :], b1_d[:, :])
            b2t = constp.tile([P, 1], FP, tag="b2")
            nc.sync.dma_start(b2t[:, :], b2_d[:, :])
            sdst = constp.tile([P, CHS], FP, tag="sdst")
            nc.sync.dma_start(sdst[:, :], dr_d[:, :])
            ddcol_t = constp.tile([P, n_blocks], FP, tag="ddcol")
            nc.sync.dma_start(ddcol_t[:, :], ddcol_d[:, :])
            snorm = constp.tile([P, CHS], FP, tag="snorm")
            nc.sync.dma_start(snorm[:, :], nm_d[:, :])
            t2keep = constp.tile([P, n_blocks, P], BF, tag="t2keep")
            idx1_ts = []
            for j in range(NBANKP):
                it = constp.tile([P, max(Cju[j], 1) * 8], I16, tag=f"idx1{j}", name=f"idx1t{j}")
                nc.sync.dma_start(it[:, :], idx1_ds[j][:, :])
                idx1_ts.append(it)
            idx2_ts = []
            for j in range(NBANK):
                it = constp.tile([P, max(Cj[j], 1) * 8], I16, tag=f"idx2{j}", name=f"idx2t{j}")
                nc.sync.dma_start(it[:, :], idx2_ds[j][:, :])
                idx2_ts.append(it)

            qrot = [0]

            def s_tile(ch):
                s = sp.tile([P, P], BF, tag="s", name="s")
                nc.vector.tensor_scalar(
                    out=s[:, :],
                    in0=iota_t[:, :],
                    scalar1=sdst[:, ch : ch + 1],
                    scalar2=snorm[:, ch : ch + 1],
                    op0=ALU.is_equal,
                    op1=ALU.mult,
                )
                return s

            def make_win_mgr(pool, tag_pfx, src_d, bank_rows, n_src_rows,
                             nbanks, idx_ts, Cj_, elem):
                """Window-packed gathers: per bank, chunks stream in windows
                of MAXC chunks; calls emitted lazily on demand."""
                wins = [dict() for _ in range(nbanks)]

                def ensure(j, ch_end):
                    while len(wins[j]) * MAXC < ch_end:
                        w = len(wins[j])
                        c0 = w * MAXC
                        cc = min(MAXC, Cj_[j] - c0)
                        mt = pool.tile([P, MAXC, elem], BF, tag=f"{tag_pfx}{j}",
                                       name=f"{tag_pfx}t{j}")
                        lo = j * bank_rows
                        hi = min((j + 1) * bank_rows, n_src_rows)
                        nc.gpsimd.dma_gather(
                            out_ap=mt[:, 0:cc, :],
                            in_ap=src_d[lo:hi, :],
                            idxs_ap=idx_ts[j][:, c0 * 8 : (c0 + cc) * 8],
                            num_idxs=cc * P,
                            num_idxs_reg=cc * P,
                            elem_size=elem,
                            queue_num=qrot[0] % NSWQ,
                        )
                        qrot[0] += 1
                        wins[j][w] = mt
                    return wins[j]

                return ensure

            # ---------------- layer 1 (paired gathers) ----------------
            ensure1 = make_win_mgr(mp1, "w1b", xp_d, NPAIR_BANK, n_pairs,
                                   NBANKP, idx1_ts, Cju, 2 * P)
            for g in range(ngroups):
                blocks = list(range(g * GB, min((g + 1) * GB, n_blocks)))
                nb = len(blocks)
                W = nb * P
                xot = xs.tile([P, GB * P], BF, tag="xot")
                nc.sync.dma_start(xot[:, :W], xots_d[:, g * GB * P : g * GB * P + W])
                aggsb = aggs.tile([P, GB * P], R32, tag="aggsb")
                for i, b in enumerate(blocks):
                    ps = aggp.tile([P, P], FP, tag="agg")
                    nmm = 1 + sum(
                        (ucol[b][j][k] >= 0) + (vcol[b][j][k] >= 0)
                        for j in range(NBANKP)
                        for k in range(Ku[b][j])
                    )
                    nc.tensor.matmul(
                        out=ps[:, :], lhsT=identb[:, :],
                        rhs=xot[:, i * P : (i + 1) * P],
                        start=True, stop=(nmm == 1),
                    )
                    ci = 1
                    for j in range(NBANKP):
                        wd = ensure1(j, int(cumKu[j, b + 1]))
                        ch0 = int(cumKu[j, b])
                        for k in range(Ku[b][j]):
                            ch = ch0 + k
                            mt = wd[ch // MAXC]
                            sl = ch % MAXC
                            for col, h in ((ucol[b][j][k], 0), (vcol[b][j][k], 1)):
                                if col < 0:
                                    continue
                                s = s_tile(int(col))
                                nc.tensor.matmul(
                                    out=ps[:, :],
                                    lhsT=mt[:, sl, h * P : (h + 1) * P],
                                    rhs=s[:, :],
                                    start=False,
                                    stop=(ci == nmm - 1),
                                )
                                ci += 1
                    nc.scalar.activation(aggsb[:, i * P : (i + 1) * P], ps[:, :], AF.Copy)
                h1 = []
                for h in range(2):
                    hp = dps.tile([P, GB * P], FP, tag="big")
                    nc.tensor.matmul(
                        out=hp[:, :W], lhsT=(w1a, w1b)[h][:, :], rhs=aggsb[:, :W],
                        start=True, stop=True,
                    )
                    hb = hs.tile([P, GB * P], R32, tag=f"h1{h}")
                    nc.scalar.activation(hb[:, :W], hp[:, :W], AF.Relu, bias=b1t[:, h : h + 1])
                    h1.append(hb)
                tp_ = dps.tile([P, GB * P], FP, tag="big")
                nc.tensor.matmul(out=tp_[:, :W], lhsT=w2a[:, :], rhs=h1[0][:, :W], start=True, stop=False)
                nc.tensor.matmul(out=tp_[:, :W], lhsT=w2b[:, :], rhs=h1[1][:, :W], start=False, stop=True)
                t2b = t2s.tile([P, GB * P], BF, tag="t2b")
                nc.scalar.activation(t2b[:, :W], tp_[:, :W], AF.Copy)
                for i, b in enumerate(blocks):
                    tpps = tpp.tile([P, P], BF, tag="tp_b", bufs=2)
                    nc.tensor.transpose(out=tpps[:, :], in_=t2b[:, i * P : (i + 1) * P], identity=identb[:, :])
                    nc.scalar.activation(t2keep[:, b, :], tpps[:, :], AF.Copy)
                    r0 = b * P
                    nc.sync.dma_start(t2_own[r0 : r0 + P, :], t2keep[:, b, :])

            # ---------------- exchange t2 shards ----------------
            if timing_variant:
                nc.sync.dma_start(t2_cat[0:OWN, :], t2_own[:, :])
            else:
                nc.gpsimd.collective_compute(
                    "AllGather",
                    ALU.bypass,
                    replica_groups=[list(range(NCORES))],
                    ins=[t2_own[:, :]],
                    outs=[t2_cat[:, :]],
                )

            # ---------------- layer 2 ----------------
            ensure2 = make_win_mgr(mp2, "w2b", t2_cat, BANK, n_rows,
                                   NBANK, idx2_ts, Cj, P)
            for g in range(ngroups):
                blocks = list(range(g * GB, min((g + 1) * GB, n_blocks)))
                for i, b in enumerate(blocks):
                    ps = aggp.tile([P, P], FP, tag="agg")
                    nchunks = 1 + sum(Kbj[b])
                    ddb = ddp.tile([P, P], BF, tag="ddb")
                    nc.vector.tensor_scalar(
                        out=ddb[:, :],
                        in0=identb[:, :],
                        scalar1=ddcol_t[:, b : b + 1],
                        scalar2=None,
                        op0=ALU.mult,
                    )
                    nc.tensor.matmul(
                        out=ps[:, :], lhsT=t2keep[:, b, :], rhs=ddb[:, :],
                        start=True, stop=(nchunks == 1),
                    )
                    ci = 1
                    for j in range(NBANK):
                        if Kbj[b][j] == 0:
                            continue
                        wd = ensure2(j, int(cumK[j, b + 1]))
                        ch0 = int(cumK[j, b])
                        for k in range(Kbj[b][j]):
                            ch = CHS1 + int(chunk_base[b, j]) + k
                            gch = ch0 + k
                            mt = wd[gch // MAXC]
                            sl = gch % MAXC
                            s = s_tile(ch)
                            nc.tensor.matmul(
                                out=ps[:, :],
                                lhsT=mt[:, sl, :],
                                rhs=s[:, :],
                                start=False,
                                stop=(ci == nchunks - 1),
                            )
                            ci += 1
                    z = zs.tile([P, P], FP, tag="z")
                    nc.scalar.activation(z[:, :], ps[:, :], AF.Relu, bias=b2t[:, 0:1])
                    tpps = tpp.tile([P, P], FP, tag="tp")
                    nc.tensor.transpose(out=tpps[:, :], in_=z[:, :], identity=ident[:, :])
                    tsb = outs.tile([P, P], FP, tag="tsb_f")
                    nc.scalar.activation(tsb[:, :], tpps[:, :], AF.Copy)
                    r0 = b * P
                    nc.sync.dma_start(out_d[r0 : r0 + P, :], tsb[:, :])

    nc.compile()
    return nc


def _pair_core(src_c, dst_rel_blk, blk, n_blocks, n_rows, rng):
    """Greedy pairing for one core's layer-1 edges.

    src_c: global source node per edge; blk: dst block per edge;
    dst_rel_blk: dst % 128 per edge.  Returns (perm, cells) where
    cells[b][j] = list of slot tuples (entry_local, du, nu_idx, dv, nv_idx)
    with nu_idx/nv_idx = edge indices (for norm lookup), -1 if absent.
    """
    E = src_c.size
    order = np.argsort(blk, kind="stable")
    bounds = np.searchsorted(blk[order], np.arange(n_blocks + 1))

    partner = np.full(n_rows, -1, np.int64)
    # edge lists per (block): process greedily
    # pair_of_edge[e] = paired edge index or -1
    mate = np.full(E, -1, np.int64)
    for b in range(n_blocks):
        eidx = order[bounds[b] : bounds[b + 1]]
        if eidx.size == 0:
            continue
        ss = src_c[eidx]
        su = np.argsort(ss, kind="stable")
        eidx = eidx[su]
        ss = ss[su]
        uniq, starts, cnt = np.unique(ss, return_index=True, return_counts=True)
        pidx = {u: i for i, u in enumerate(uniq)}
        taken = np.zeros(uniq.size, np.int64)  # how many edges of uniq[i] consumed
        # 1) existing partners co-present
        for i, u in enumerate(uniq):
            pu = partner[u]
            if pu >= 0 and pu in pidx:
                j = pidx[pu]
                if j <= i:
                    continue  # handle once (i<j)
                m = min(cnt[i], cnt[j])
                for k in range(m):
                    e1 = eidx[starts[i] + k]
                    e2 = eidx[starts[j] + k]
                    mate[e1] = e2
                    mate[e2] = e1
                taken[i] += m
                taken[j] += m
        # 2) match unmatched sources with each other
        rem = [i for i in range(uniq.size) if taken[i] < cnt[i] and partner[uniq[i]] < 0]
        for a in range(0, len(rem) - 1, 2):
            i, j = rem[a], rem[a + 1]
            u, v = uniq[i], uniq[j]
            partner[u] = v
            partner[v] = u
            m = min(cnt[i] - taken[i], cnt[j] - taken[j])
            for k in range(m):
                e1 = eidx[starts[i] + taken[i] + k]
                e2 = eidx[starts[j] + taken[j] + k]
                mate[e1] = e2
                mate[e2] = e1
            taken[i] += m
            taken[j] += m
    # table layout: matched pairs share an entry; everything else fills up
    pos = np.full(n_rows, -1, np.int64)  # node -> table row position
    entries = []
    done = np.zeros(n_rows, bool)
    for u in range(n_rows):
        if done[u]:
            continue
        v = partner[u]
        if v >= 0:
            entries.append((u, v))
            done[u] = done[v] = True
        else:
            entries.append((u, -1))
            done[u] = True
    # second pass: merge singleton entries pairwise to keep table = n_rows
    merged = []
    half_open = -1
    for (u, v) in entries:
        if v >= 0:
            merged.append((u, v))
        elif half_open < 0:
            half_open = u
        else:
            merged.append((half_open, u))
            half_open = -1
    if half_open >= 0:
        merged.append((half_open, -1))
    # bank-aware placement: fill bank-0 cells up to a 512-slot target so
    # per-(block,bank) chunk counts hit the ceil-128 boundaries with minimal
    # padding (bank 1 takes the remainder).
    node_blocks = {}
    for e in range(E):
        u = int(src_c[e])
        b = int(blk[e])
        d = node_blocks.setdefault(u, {})
        d[b] = d.get(b, 0) + 1
    merged = [merged[i] for i in rng.permutation(len(merged))]
    CAP0 = 4 * P
    used0 = np.zeros(n_blocks, np.int64)
    bank0, bank1 = [], []
    for ent in merged:
        u, v = ent
        slots = {}
        for node in (u, v):
            if node < 0:
                continue
            for b, ccnt in node_blocks.get(node, {}).items():
                slots[b] = max(slots.get(b, 0), ccnt)
        fits0 = (
            len(bank0) < NPAIR_BANK
            and all(used0[b] + cS <= CAP0 for b, cS in slots.items())
        )
        if fits0:
            bank0.append(ent)
            for b, cS in slots.items():
                used0[b] += cS
        elif len(bank1) < NPAIR_BANK:
            bank1.append(ent)
        else:
            bank0.append(ent)
    for k, (u, v) in enumerate(bank0):
        pos[u] = 2 * k
        if v >= 0:
            pos[v] = 2 * k + 1
    for k0, (u, v) in enumerate(bank1):
        k = NPAIR_BANK + k0
        pos[u] = 2 * k
        if v >= 0:
            pos[v] = 2 * k + 1
    return mate, pos


def _build_pair_layout(src, dst, norm_e, n_rows, OWN, n_blocks, ncores, rng):
    """Full per-core pairing + slot layout.

    Returns:
      tables: list per core of perm_pos [n_rows] (node -> paired-table row)
      slotinfo: dict with per-core per-(block,bank) slot arrays
    """
    results = []
    for c in range(ncores):
        sel = (dst // OWN) == c
        s_c = src[sel]
        d_c = dst[sel]
        n_c = norm_e[sel]
        blk = (d_c % OWN) // P
        drel = d_c % P
        mate, pos = _pair_core(s_c, drel, blk, n_blocks, n_rows, rng)
        # build slot lists per (block, bank)
        entry = pos[s_c] // 2          # pair-entry per edge
        half = pos[s_c] % 2            # which half of the entry
        bank = entry // NPAIR_BANK
        cells = [[[] for _ in range(NBANKP)] for _ in range(n_blocks)]
        E = s_c.size
        used = np.zeros(E, bool)
        eorder = np.lexsort((s_c, blk))
        for e in eorder:
            if used[e]:
                continue
            m = mate[e]
            b = blk[e]
            j = bank[e]
            if m >= 0 and not used[m]:
                # paired slot: e and m share entry[e]==entry[m]
                assert entry[m] == entry[e] and blk[m] == b
                used[e] = used[m] = True
                if half[e] == 0:
                    cells[b][j].append((entry[e] - j * NPAIR_BANK,
                                        drel[e], n_c[e], drel[m], n_c[m]))
                else:
                    cells[b][j].append((entry[e] - j * NPAIR_BANK,
                                        drel[m], n_c[m], drel[e], n_c[e]))
            else:
                used[e] = True
                if half[e] == 0:
                    cells[b][j].append((entry[e] - j * NPAIR_BANK,
                                        drel[e], n_c[e], -1, 0.0))
                else:
                    cells[b][j].append((entry[e] - j * NPAIR_BANK,
                                        -1, 0.0, drel[e], n_c[e]))
        # sort each cell: both -> v-only -> u-only
        for b in range(n_blocks):
            for j in range(NBANKP):
                cells[b][j].sort(
                    key=lambda t: 0 if (t[1] >= 0 and t[3] >= 0) else (1 if t[3] >= 0 else 2)
                )
        results.append((pos, cells))
    return results


def _preprocess(x, edge_index, W1, b1, W2, b2):
    N = x.shape[0]
    OWN = int(math.ceil(N / (NCORES * P))) * P
    n_blocks = OWN // P
    n_rows = NCORES * OWN
    NBLK = NCORES * n_blocks
    BFNP = ml_dtypes.bfloat16

    src = np.asarray(edge_index[0], np.int64)
    dst = np.asarray(edge_index[1], np.int64)

    deg = (np.bincount(dst, minlength=N) + 1).astype(np.float64)
    dinv = (1.0 / np.sqrt(deg)).astype(np.float32)
    norm_e = dinv[src] * dinv[dst]

    # ================= layer 1: paired layout =================
    rng = np.random.default_rng(12345)
    pres = _build_pair_layout(src, dst, norm_e, n_rows, OWN, n_blocks, NCORES, rng)

    # shared chunk counts (max over cores)
    Ku = np.zeros((n_blocks, NBANKP), np.int64)
    for c in range(NCORES):
        _, cells = pres[c]
        for b in range(n_blocks):
            for j in range(NBANKP):
                Ku[b, j] = max(Ku[b, j], (len(cells[b][j]) + P - 1) // P)
    maxK = int(Ku.max())
    # per-chunk half flags (OR over cores)
    has_u = np.zeros((n_blocks, NBANKP, maxK), bool)
    has_v = np.zeros((n_blocks, NBANKP, maxK), bool)
    for c in range(NCORES):
        _, cells = pres[c]
        for b in range(n_blocks):
            for j in range(NBANKP):
                cell = cells[b][j]
                for k in range(Ku[b, j]):
                    seg = cell[k * P : (k + 1) * P]
                    if any(t[1] >= 0 for t in seg):
                        has_u[b, j, k] = True
                    if any(t[3] >= 0 for t in seg):
                        has_v[b, j, k] = True
    # S-column assignment (shared)
    ucol = -np.ones((n_blocks, NBANKP, maxK), np.int64)
    vcol = -np.ones((n_blocks, NBANKP, maxK), np.int64)
    colc = 0
    for b in range(n_blocks):
        for j in range(NBANKP):
            for k in range(Ku[b, j]):
                if has_u[b, j, k]:
                    ucol[b, j, k] = colc
                    colc += 1
                if has_v[b, j, k]:
                    vcol[b, j, k] = colc
                    colc += 1
    CHS1 = colc
    cumKu = np.zeros((NBANKP, n_blocks + 1), np.int64)
    for j in range(NBANKP):
        for b in range(n_blocks):
            cumKu[j, b + 1] = cumKu[j, b] + Ku[b, j]
    Cju = [int(cumKu[j, n_blocks]) for j in range(NBANKP)]

    # per-core staging: idx streams, S tables, paired x tables
    stage1_dst = np.zeros((NCORES, P, CHS1), np.float32)
    stage1_nrm = np.zeros((NCORES, P, CHS1), np.float32)
    idx1 = [np.zeros((NCORES, Cju[j] * P), np.int16) for j in range(NBANKP)]
    xpairs = np.zeros((NCORES, n_rows // 2, 2 * D_IN), BFNP)
    xpad = np.zeros((n_rows, D_IN), np.float32)
    xpad[:N] = np.asarray(x, np.float32)
    for c in range(NCORES):
        pos, cells = pres[c]
        posinv = np.empty(n_rows, np.int64)
        posinv[pos] = np.arange(n_rows)
        xpairs[c] = xpad[posinv].astype(BFNP).reshape(n_rows // 2, 2 * D_IN)
        for b in range(n_blocks):
            for j in range(NBANKP):
                cell = cells[b][j]
                base = int(cumKu[j, b]) * P
                for k in range(Ku[b, j]):
                    seg = cell[k * P : (k + 1) * P]
                    cu = ucol[b, j, k]
                    cv = vcol[b, j, k]
                    for p, t in enumerate(seg):
                        ent, du, nu, dv, nv = t
                        idx1[j][c, base + k * P + p] = ent
                        if du >= 0 and cu >= 0:
                            stage1_dst[c, p, cu] = du
                            stage1_nrm[c, p, cu] = nu
                        if dv >= 0 and cv >= 0:
                            stage1_dst[c, p, cv] = dv
                            stage1_nrm[c, p, cv] = nv

    # idx wrap: [cores, cj*P] -> 16-partition wrapped, replicated to 128 rows
    def wrap_idx(flat, cj):
        if cj == 0:
            return np.zeros((NCORES, P, 8), np.int16)
        w = flat.reshape(NCORES, cj * 8, 16).transpose(0, 2, 1)
        return np.ascontiguousarray(np.tile(w, (1, 8, 1)))

    idx1_stages = [wrap_idx(idx1[j], Cju[j]) for j in range(NBANKP)]

    # ================= layer 2: banked layout (unpaired) =================
    gblk = dst // P
    bank = src // BANK
    cell = gblk * NBANK + bank
    order = np.argsort(cell, kind="stable")
    s_src = src[order]
    s_dst = dst[order]
    s_norm = norm_e[order].astype(np.float32)
    s_cell = cell[order]
    s_bank = s_src // BANK

    counts = np.bincount(s_cell, minlength=NBLK * NBANK)
    percell = counts.reshape(NCORES, n_blocks, NBANK)
    Kbj = np.ceil(percell.max(axis=0) / P).astype(np.int64)
    caps = Kbj * P
    cell_off = np.concatenate(([0], np.cumsum(caps.ravel())))[:-1].reshape(n_blocks, NBANK)
    TOT = int(caps.sum())
    CH2 = int(Kbj.sum())

    starts = np.concatenate(([0], np.cumsum(counts)))[:-1]
    pos_e = np.arange(s_dst.size) - starts[s_cell]
    core = (gblk[order] // n_blocks).astype(np.int64)
    lblk = (gblk[order] % n_blocks).astype(np.int64)
    slot = cell_off[lblk, s_bank] + pos_e

    arr_rel = np.zeros((NCORES, TOT), np.int16)
    arr_dst = np.zeros((NCORES, TOT), np.float32)
    arr_nrm = np.zeros((NCORES, TOT), np.float32)
    arr_rel[core, slot] = (s_src - s_bank * BANK).astype(np.int16)
    arr_dst[core, slot] = (s_dst % P).astype(np.float32)
    arr_nrm[core, slot] = s_norm

    stage2_dst = arr_dst.reshape(NCORES, CH2, P).transpose(0, 2, 1)
    stage2_nrm = arr_nrm.reshape(NCORES, CH2, P).transpose(0, 2, 1)

    chunk_bank = np.repeat(np.tile(np.arange(NBANK), n_blocks), Kbj.ravel())
    rel3 = arr_rel.reshape(NCORES, CH2, P)
    idx2_stages = []
    for j in range(NBANK):
        selj = chunk_bank == j
        cj = int(selj.sum())
        idx2_stages.append(wrap_idx(rel3[:, selj, :].reshape(NCORES, cj * P), cj))

    # combined S tables
    stage_dst = np.ascontiguousarray(
        np.concatenate([stage1_dst, stage2_dst], axis=2)
    )
    stage_nrm = np.ascontiguousarray(
        np.concatenate([stage1_nrm, stage2_nrm], axis=2)
    )

    # dense self-loop tables
    dinv2 = np.zeros(n_rows, np.float64)
    dinv2[:N] = 1.0 / deg
    x64 = np.asarray(x, np.float64)
    xots = np.zeros((NCORES, P, OWN), BFNP)
    ddcol = np.zeros((NCORES, P, n_blocks), np.float32)
    for c in range(NCORES):
        r0, r1 = c * OWN, min((c + 1) * OWN, N)
        if r1 > r0:
            sc = (x64[r0:r1] * dinv2[r0:r1, None]).astype(BFNP)
            xots[c, :, : r1 - r0] = sc.T
        ddcol[c] = dinv2[c * OWN : (c + 1) * OWN].astype(np.float32).reshape(n_blocks, P).T

    w1 = np.ascontiguousarray(np.asarray(W1, np.float32))
    w2 = np.ascontiguousarray(np.asarray(W2, np.float32))
    b1h = np.ascontiguousarray(np.asarray(b1, np.float32).reshape(2, P).T)
    b2c = np.ascontiguousarray(np.asarray(b2, np.float32).reshape(P, 1))
    iota = np.ascontiguousarray(np.tile(np.arange(P), (P, 1)).astype(BFNP))
    identb = np.ascontiguousarray(np.eye(P).astype(BFNP))

    in_maps = []
    for c in range(NCORES):
        m = {
            "xpair": np.ascontiguousarray(xpairs[c]),
            "xots": np.ascontiguousarray(xots[c]),
            "ddcol": np.ascontiguousarray(ddcol[c]),
            "w1": w1,
            "w2": w2,
            "b1h": b1h,
            "b2c": b2c,
            "iota": iota,
            "identb": identb,
            "dst_rel": np.ascontiguousarray(stage_dst[c]),
            "norm": np.ascontiguousarray(stage_nrm[c]),
        }
        for j in range(NBANKP):
            m[f"idx1_{j}"] = idx1_stages[j][c]
        for j in range(NBANK):
            m[f"idx2_{j}"] = idx2_stages[j][c]
        in_maps.append(m)

    meta = {
        "OWN": OWN,
        "n_blocks": n_blocks,
        "Ku": [list(map(int, r)) for r in Ku],
        "ucol": ucol,
        "vcol": vcol,
        "CHS1": int(CHS1),
        "Kbj": [list(map(int, r)) for r in Kbj],
    }
    digest = hashlib.sha1(
        b"|".join(
            [
                np.asarray(Ku).tobytes(),
                ucol.tobytes(),
                vcol.tobytes(),
                np.asarray(Kbj).tobytes(),
                str(OWN).encode(),
            ]
        )
    ).hexdigest()
    return in_maps, N, meta, digest


def run(x, edge_index, W1, b1, W2, b2, trace=False):
    from concourse.bass_utils import run_bass_kernel_spmd

    in_maps, N, meta, digest = _preprocess(x, edge_index, W1, b1, W2, b2)
    nc = _CACHE.get(digest)
    if nc is None:
        nc = _build(meta)
        _CACHE[digest] = nc

    res = run_bass_kernel_spmd(nc, in_maps, core_ids=list(range(NCORES)), trace=trace)
    out = np.concatenate([res.results[c]["out"] for c in range(NCORES)], axis=0)[:N]
    return np.ascontiguousarray(out.astype(np.float32)), res


def kernel(x, edge_index, W1, b1, W2, b2):
    out, _ = run(x, edge_index, W1, b1, W2, b2, trace=False)
    return out


def estimate_time_ns(np_inputs):
    """Cost-model (TimelineSim) per-core time estimate + AllGather table cost."""
    from concourse.timeline_sim import TimelineSim

    in_maps, N, meta, digest = _preprocess(**np_inputs)
    key = ("timing", digest)
    nc = _CACHE.get(key)
    if nc is None:
        nc = _build(meta, timing_variant=True)
        _CACHE[key] = nc
    ts = TimelineSim(nc)
    t = ts.simulate()
    AG_NS = 35000.0  # 8-core AllGather @ ~6.4MB/rank (measured-latency table)
    return t + AG_NS
